# revision 39
# baseline (speedup 1.0000x reference)
"""Trainium2 Bass kernel: CustomFlashAttention (B=1, S=2048, D=2048, H=16, Hd=128).

Sharding (Megatron tensor-parallel over heads, 8 NeuronCores):
  - each core owns 2 heads (256 feature dims)
  - w_q/w_k/w_v column-parallel (pre-transposed + sliced on host)
  - w_o row-parallel; cores produce fp16 partial outputs, host sums them

Device layout: activations feature-major ([feat, seq]) so every contraction
lands on SBUF partitions with zero on-device transposes. Scores are computed
transposed sT[k, q]; softmax runs without max-subtraction (scores ~ N(0,1)).

Changes vs the 257us baseline (measured 207-209us, rel err 8.1e-4):
  - denominators: exp'd tiles accumulate on the vector engine in fp16 (one add
    per 2-k-tile group; group 0's exp writes the accumulator directly); a
    single pair of ones-matmuls per (chunk, head) does the cross-partition
    sum (was: 16 ones-matmuls per pair = ~27us of PE).
  - reciprocal via the custom-DVE reciprocal_approx_fast straight out of PSUM
    (was: 3.3us DVE reciprocal + copy on the critical path -> 2.3us PE stalls
    at every pair boundary + HAM re-throttles).
  - software-pipelined attention: scores for group g+1 issue before the
    PV matmuls of group g, so the PE never waits on the scalar-engine exp.
  - filler matmuls (deferred q projections + out-projections of earlier
    chunks) rebalanced to exactly 3 per 2-k-tile group in every pair; ph3
    casts split vector/scalar so neither engine exceeds the pair wall.
  - warmup burst of matmuls at t=0 trips the HAM clock gate early
    (PE cold 1.2GHz -> 2.4GHz by ~12us instead of 17us).
  - input DMAs striped across the 3 DMA-capable queues (sync/gpsimd/scalar)
    in exactly the order phase 1 consumes them (HBM read sustains only
    ~280GB/s, so arrival order is the phase-1 pacing constraint); chunk-0's
    q projection runs at the end of phase 1 so wq can arrive after x.
  - output partials in fp16 (halves output DMA; host sums in float64); final
    out-projection as 8 double-units with 256KB DMAs alternating two queues,
    head-0 partials pre-issued to cover the last normalization's latency.
"""

import sys
from contextlib import ExitStack

import numpy as np

if "/opt/trn_rl_repo" not in sys.path:
    sys.path.insert(0, "/opt/trn_rl_repo")

import concourse.bass as bass  # noqa: F401
import concourse.tile as tile
from concourse import bacc, mybir
from concourse.bass_utils import run_bass_kernel_spmd

P = 128                      # SBUF partitions
S = 2048                     # sequence length
D = 2048                     # hidden dim
H = 16                       # heads
HD = 128                     # head dim
NCORES = 8
HPC = H // NCORES            # heads per core = 2
HDC = HPC * HD               # feature dims per core = 256
DT = D // P                  # 16 contraction tiles
NCH = 4                      # seq chunks
CH = S // NCH                # 512
KT = S // P                  # 16 key tiles
NG = KT // 2                 # 8 groups of 2 key tiles
SCALE = 1.0 / float(np.sqrt(HD))
WARM_MM = 8                  # warmup matmuls to trip the HAM clock gate
WARM_N = 448                 # wide enough that PE duty cycle counts as busy

f32 = mybir.dt.float32
f16 = mybir.dt.float16

_CACHE = {}
LAST_RESULT = None


def _build_nc():
    nc = bacc.Bacc("TRN2", target_bir_lowering=False, debug=False, num_devices=NCORES)

    xT = nc.dram_tensor("xT", [D, S], f16, kind="ExternalInput").ap()
    wqT = nc.dram_tensor("wqT", [D, HDC], f16, kind="ExternalInput").ap()
    wkT = nc.dram_tensor("wkT", [D, HDC], f16, kind="ExternalInput").ap()
    wvT = nc.dram_tensor("wvT", [D, HDC], f16, kind="ExternalInput").ap()
    woT = nc.dram_tensor("woT", [HDC, D], f16, kind="ExternalInput").ap()
    outT = nc.dram_tensor("outT", [D, S], f16, kind="ExternalOutput").ap()

    xT_r = xT.rearrange("(dt p) s -> dt p s", p=P)       # [16, 128, 2048]
    out_r = outT.rearrange("(ot p) s -> ot p s", p=P)    # [16, 128, 2048]

    with ExitStack() as ctx:
        tc = ctx.enter_context(tile.TileContext(nc))

        singles = ctx.enter_context(tc.tile_pool(name="singles", bufs=1))
        ppool = ctx.enter_context(tc.tile_pool(name="pt", bufs=4))
        accpool = ctx.enter_context(tc.tile_pool(name="acc", bufs=2))
        rspool = ctx.enter_context(tc.tile_pool(name="rs", bufs=2))
        obpool = ctx.enter_context(tc.tile_pool(name="ob", bufs=6))
        p1_ctx = ExitStack()
        k_ps = p1_ctx.enter_context(tc.tile_pool(name="kps", bufs=2, space="PSUM"))
        q_ps = p1_ctx.enter_context(tc.tile_pool(name="qps", bufs=2, space="PSUM"))
        v_ps = p1_ctx.enter_context(tc.tile_pool(name="vps", bufs=4, space="PSUM"))

        # Persistent SBUF tensors
        x_sb = singles.tile([P, DT, S], f16, tag="x")
        wq_sb = singles.tile([P, DT, HDC], f16, tag="wq")
        wk_sb = singles.tile([P, DT, HDC], f16, tag="wk")
        wv_sb = singles.tile([P, DT, HDC], f16, tag="wv")
        wo_sb = singles.tile([P, HDC // P, D], f16, tag="wo")
        qT_sb = singles.tile([P, HPC, S], f16, tag="qT")
        kT_sb = singles.tile([P, HPC, S], f16, tag="kT")
        v_sb = singles.tile([P, KT, HDC], f16, tag="v")
        oT_sb = singles.tile([P, HPC, S], f16, tag="oT")
        ones = singles.tile([P, WARM_N], f16, tag="ones")

        nc.vector.memset(ones, 1.0)

        # ---------- input DMA, ordered exactly as phase 1 consumes it ----------
        # HBM->SBUF sustains only ~300 GB/s, so arrival order is everything:
        # phase 1 runs K/V for chunks 0..3 (needs wk/wv/x only), then chunk-0's
        # Q block (needs wq). wo is needed ~40us later still.
        wk_r = wkT.rearrange("(dt p) h -> p dt h", p=P)
        wv_r = wvT.rearrange("(dt p) h -> p dt h", p=P)
        xT_r2 = xT.rearrange("(dt2 two p) s -> dt2 p two s", two=2, p=P)  # [8,128,2,S]
        wq_r2 = wqT.rearrange("(dt2 two p) h -> dt2 p two h", two=2, p=P)
        queues = [nc.sync, nc.gpsimd, nc.scalar]
        for d in range(DT):
            q = queues[d % 3]
            q.dma_start(out=wk_sb[:, d, :], in_=wk_r[:, d, :])
            q.dma_start(out=x_sb[:, d, 0:CH], in_=xT_r[d][:, 0:CH])
            q.dma_start(out=wv_sb[:, d, :], in_=wv_r[:, d, :])
        qi = 0
        for c in range(1, NCH):
            csl = slice(c * CH, (c + 1) * CH)
            for d2 in range(DT // 2):
                queues[qi % 3].dma_start(
                    out=x_sb[:, 2 * d2:2 * d2 + 2, csl], in_=xT_r2[d2][:, :, csl])
                qi += 1
            if c == 2:
                for d2 in range(DT // 2):
                    queues[qi % 3].dma_start(
                        out=wq_sb[:, 2 * d2:2 * d2 + 2, :], in_=wq_r2[d2])
                    qi += 1
        nc.gpsimd.dma_start(out=wo_sb, in_=woT.rearrange("(it p) o -> p it o", p=P))

        # ---------- warmup: trip the HAM clock gate while DMAs land ----------
        warm = k_ps.tile([P, CH], f32, tag="pk", name="warm")
        for _ in range(WARM_MM):
            nc.tensor.matmul(warm[:, 0:WARM_N], lhsT=ones[:, 0:P],
                             rhs=ones[:, 0:WARM_N], start=True, stop=True)

        # q projections for chunks 1-3 are deferred into the attention stream
        # as filler matmuls. Chunk 0's q runs as a block at the end of phase 1
        # so wq can arrive after chunk 1's x without stalling the K/V d-loops
        # (the HBM read rate, ~280 GB/s, is the binding constraint early on).
        # ---------- Phase 1: K/V for all chunks, then q for chunk 0 ----------
        def emit_kv(c):
            csl = slice(c * CH, (c + 1) * CH)
            pk = [k_ps.tile([P, CH], f32, tag="pk", name=f"pk{c}_{i}") for i in range(HPC)]
            pv = [v_ps.tile([P, HDC], f32, tag="pv", name=f"pv{c}_{i}") for i in range(4)]
            for d in range(DT):
                first, last = (d == 0), (d == DT - 1)
                for h in range(HPC):
                    nc.tensor.matmul(
                        pk[h],
                        lhsT=wk_sb[:, d, h * HD:(h + 1) * HD],
                        rhs=x_sb[:, d, csl], start=first, stop=last,
                    )
                for st in range(4):
                    nc.tensor.matmul(
                        pv[st],
                        lhsT=x_sb[:, d, c * CH + st * P:c * CH + (st + 1) * P],
                        rhs=wv_sb[:, d, :],
                        start=first, stop=last,
                    )
            for h in range(HPC):
                nc.vector.tensor_copy(kT_sb[:, h, csl], pk[h])
            for st in range(4):
                nc.vector.tensor_copy(v_sb[:, c * 4 + st, :], pv[st])

        for c in range(NCH):
            emit_kv(c)
        pq = [q_ps.tile([P, CH], f32, tag="pq", name=f"pq0_{i}") for i in range(HPC)]
        for d in range(DT):
            for h in range(HPC):
                nc.tensor.matmul(
                    pq[h],
                    lhsT=wq_sb[:, d, h * HD:(h + 1) * HD],
                    rhs=x_sb[:, d, 0:CH], start=(d == 0), stop=(d == DT - 1),
                )
        for h in range(HPC):
            nc.vector.tensor_copy(qT_sb[:, h, 0:CH], pq[h])

        p1_ctx.close()  # release phase-1 PSUM banks

        # ---------- phase-2 PSUM pools (8 banks total) ----------
        sc_ps = ctx.enter_context(tc.tile_pool(name="scps", bufs=2, space="PSUM"))  # 2x2 banks
        o_ps = ctx.enter_context(tc.tile_pool(name="ops", bufs=2, space="PSUM"))    # 2x1 bank
        aux_ps = ctx.enter_context(tc.tile_pool(name="auxps", bufs=2, space="PSUM"))  # 2x1 bank

        # ---------- filler emission units ----------
        def make_defq_units(c, h):
            """16 single-matmul units projecting q for (c, h); last drains PSUM.

            The PSUM tile is allocated lazily at the first emitted matmul so
            the aux-pool rotation follows emission order.
            """
            csl = slice(c * CH, (c + 1) * CH)
            state = {}

            def unit(d):
                def emit():
                    if "pq" not in state:
                        state["pq"] = aux_ps.tile(
                            [P, CH], f32, tag="aux", name=f"dpq{c}_{h}")
                    pq = state["pq"]
                    nc.tensor.matmul(
                        pq,
                        lhsT=wq_sb[:, d, h * HD:(h + 1) * HD],
                        rhs=x_sb[:, d, csl],
                        start=(d == 0), stop=(d == DT - 1),
                    )
                    if d == DT - 1:
                        nc.vector.tensor_copy(qT_sb[:, h, csl], pq)
                return emit

            return [unit(d) for d in range(DT)]

        def make_ph3_units(c):
            """Out-projection units for chunk c: each is 2 matmuls + cast + DMA.

            Counts as 2 filler matmuls. Every 3rd cast runs on scalar to keep
            the vector engine under the pair wall; DMAs go on sync.
            """
            csl = slice(c * CH, (c + 1) * CH)

            def unit(ot):
                def emit():
                    pout = aux_ps.tile([P, CH], f32, tag="aux", name=f"pout{c}_{ot}")
                    for di in range(HDC // P):
                        nc.tensor.matmul(
                            pout,
                            lhsT=wo_sb[:, di, ot * P:(ot + 1) * P],
                            rhs=oT_sb[:, di, csl],
                            start=(di == 0), stop=(di == HDC // P - 1),
                        )
                    ob = obpool.tile([P, CH], f16, tag="ob", name=f"ob{c}_{ot}")
                    if ot % 3 == 2:
                        nc.scalar.copy(ob, pout)
                    else:
                        nc.vector.tensor_copy(ob, pout)
                    nc.sync.dma_start(out=out_r[ot][:, csl], in_=ob)
                return emit

            return [unit(ot) for ot in range(DT)]

        def emit_final_ph3(c):
            """Final chunk's out-projection as 8 double-units: 4 matmuls into a
            2-bank sc_ps tile, one [P,2,CH] cast (alternating vector/scalar),
            one 256KB DMA (alternating sync/gpsimd). Minimizes the end-of-
            kernel descriptor backlog and drain."""
            csl = slice(c * CH, (c + 1) * CH)
            out_r2 = outT.rearrange("(og two p) s -> og p two s", two=2, p=P)
            # head-0 partials of the first two units can run before the last
            # pair's oT normalization (which only head-1's matmul needs)
            head0 = {}
            for og in range(2):
                pout2 = sc_ps.tile([P, 2, CH], f32, tag="psc", name=f"fpo{og}")
                head0[og] = pout2
                for j in range(2):
                    ot = og * 2 + j
                    nc.tensor.matmul(
                        pout2[:, j, :],
                        lhsT=wo_sb[:, 0, ot * P:(ot + 1) * P],
                        rhs=oT_sb[:, 0, csl],
                        start=True, stop=False,
                    )
            for og in range(DT // 2):
                pout2 = head0[og] if og in head0 else sc_ps.tile(
                    [P, 2, CH], f32, tag="psc", name=f"fpo{og}")
                for j in range(2):
                    ot = og * 2 + j
                    dis = range(1, HDC // P) if og in head0 else range(HDC // P)
                    for di in dis:
                        nc.tensor.matmul(
                            pout2[:, j, :],
                            lhsT=wo_sb[:, di, ot * P:(ot + 1) * P],
                            rhs=oT_sb[:, di, csl],
                            start=(di == 0), stop=(di == HDC // P - 1),
                        )
                ob2 = obpool.tile([P, 2, CH], f16, tag="ob2", name=f"ob2_{og}")
                if og == DT // 2 - 1:
                    # last unit: split cast and DMA across both engine pairs
                    # so the end-of-kernel drain is as short as possible
                    nc.vector.tensor_copy(ob2[:, 0, :], pout2[:, 0, :])
                    nc.scalar.copy(ob2[:, 1, :], pout2[:, 1, :])
                    nc.sync.dma_start(out=out_r2[og][:, 0, csl], in_=ob2[:, 0, :])
                    nc.gpsimd.dma_start(out=out_r2[og][:, 1, csl], in_=ob2[:, 1, :])
                    continue
                if og % 2 == 0:
                    nc.vector.tensor_copy(ob2, pout2)
                else:
                    nc.scalar.copy(ob2, pout2)
                dq = nc.sync if og % 2 == 0 else nc.gpsimd
                dq.dma_start(out=out_r2[og][:, :, csl], in_=ob2)

        # ---------- Phase 2: software-pipelined attention ----------
        def attention(c, h, fillers):
            """Attention for (c, h). `fillers` are single-matmul emission units
            injected 3 per group; scores for group g+1 issue before PV of g so
            the exp latency (scalar engine) is always hidden."""
            csl = slice(c * CH, (c + 1) * CH)
            po = o_ps.tile([P, CH], f32, tag="po", name=f"po{c}_{h}")
            fi = 0

            def emit_scores(g):
                psc = sc_ps.tile([P, 2, CH], f32, tag="psc", name=f"psc{c}_{h}_{g}")
                for j in range(2):
                    kj = g * 2 + j
                    nc.tensor.matmul(
                        psc[:, j, :],
                        lhsT=kT_sb[:, h, kj * P:(kj + 1) * P],
                        rhs=qT_sb[:, h, csl],
                        start=True, stop=True,
                    )
                return psc

            acc2 = accpool.tile([P, 2, CH], f16, tag="acc", name=f"acc{c}_{h}")
            psc_cur = emit_scores(0)
            for g in range(NG):
                psc_next = emit_scores(g + 1) if g + 1 < NG else None
                # independent PE work hides the exp latency
                for _ in range(3):
                    if fi < len(fillers):
                        fillers[fi]()
                        fi += 1
                # group 0's exp writes straight into the denominator
                # accumulator; later groups add onto it (fp16, vector engine)
                if g == 0:
                    pt = acc2
                else:
                    pt = ppool.tile([P, 2, CH], f16, tag="pt", name=f"pt{c}_{h}_{g}")
                nc.scalar.activation(
                    out=pt, in_=psc_cur,
                    func=mybir.ActivationFunctionType.Exp, scale=SCALE,
                )
                if g > 0:
                    nc.vector.tensor_add(acc2, acc2, pt)
                for j in range(2):
                    kj = g * 2 + j
                    nc.tensor.matmul(
                        po,
                        lhsT=v_sb[:, kj, h * HD:(h + 1) * HD],
                        rhs=pt[:, j, :],
                        start=(kj == 0), stop=(kj == KT - 1),
                    )
                psc_cur = psc_next
            while fi < len(fillers):
                fillers[fi]()
                fi += 1
            with tc.high_priority():
                pr = aux_ps.tile([P, CH], f32, tag="aux", name=f"pr{c}_{h}")
                for j in range(2):
                    nc.tensor.matmul(pr, lhsT=ones[:, 0:P], rhs=acc2[:, j, :],
                                     start=(j == 0), stop=(j == 1))
                rs = rspool.tile([P, CH], f32, tag="rs", name=f"rs{c}_{h}")
                nc.vector.reciprocal_approx_fast(rs, pr)
                nc.vector.tensor_mul(oT_sb[:, h, csl], po, rs)

        # Filler distribution: 24 single-matmul units per pair.
        # defq(c,h) must land before pair (c,h); ph3(c) after pair (c,1)+eps.
        defq = {(c, h): make_defq_units(c, h) for c in range(1, NCH) for h in range(HPC)}
        ph3 = {c: make_ph3_units(c) for c in range(NCH - 1)}

        def ph3_mm(c, lo, hi):
            """ph3 units lo..hi as a flat list of 2-matmul emitters (each unit
            counts as 2 filler slots; wrap to single-slot callables)."""
            out = []
            for u in ph3[c][lo:hi]:
                out.append(u)
                out.append(lambda: None)  # unit emits 2 MMs; burn 2 slots
            return out

        fills_for = {
            (0, 0): defq[(1, 0)] + defq[(1, 1)][:8],
            (0, 1): defq[(1, 1)][8:] + defq[(2, 0)],
            (1, 0): defq[(2, 1)] + ph3_mm(0, 0, 4),
            (1, 1): defq[(3, 0)] + ph3_mm(0, 4, 8),
            (2, 0): defq[(3, 1)] + ph3_mm(0, 8, 12),
            (2, 1): ph3_mm(0, 12, 16) + ph3_mm(1, 0, 8),
            (3, 0): ph3_mm(1, 8, 16) + ph3_mm(2, 0, 4),
            (3, 1): ph3_mm(2, 4, 16),
        }

        for c in range(NCH):
            for h in range(HPC):
                attention(c, h, fills_for[(c, h)])
        emit_final_ph3(NCH - 1)

    nc.compile()
    return nc


def _get_nc():
    if "nc" not in _CACHE:
        _CACHE["nc"] = _build_nc()
    return _CACHE["nc"]


def make_in_maps(x, w_q, w_k, w_v, w_o):
    x = np.asarray(x, dtype=np.float32).reshape(S, D)
    w_q = np.asarray(w_q, dtype=np.float32)
    w_k = np.asarray(w_k, dtype=np.float32)
    w_v = np.asarray(w_v, dtype=np.float32)
    w_o = np.asarray(w_o, dtype=np.float32)
    xT = np.ascontiguousarray(x.T).astype(np.float16)
    in_maps = []
    for c in range(NCORES):
        hs = slice(c * HDC, (c + 1) * HDC)
        in_maps.append({
            "xT": xT,
            "wqT": np.ascontiguousarray(w_q[hs, :].T).astype(np.float16),
            "wkT": np.ascontiguousarray(w_k[hs, :].T).astype(np.float16),
            "wvT": np.ascontiguousarray(w_v[hs, :].T).astype(np.float16),
            "woT": np.ascontiguousarray(w_o[:, hs].T).astype(np.float16),
        })
    return in_maps


def kernel(x, w_q, w_k, w_v, w_o):
    global LAST_RESULT
    in_maps = make_in_maps(x, w_q, w_k, w_v, w_o)
    nc = _get_nc()
    res = run_bass_kernel_spmd(nc, in_maps, core_ids=list(range(NCORES)))
    LAST_RESULT = res
    acc = np.zeros((D, S), dtype=np.float64)
    for r in res.results:
        acc += r["outT"]
    return np.ascontiguousarray(acc.T).astype(np.float32).reshape(1, S, D)
